# revision 17
# baseline (speedup 1.0000x reference)
"""Multi-head attention (16 heads, RoPE, B=2, S=2048, D=2048) on 8 trn2 cores.

Sharding: core c handles batch b=c//4 and heads [4g, 4g+4) with g=c%4.
Each core computes Q/K/V projections for its 4 heads (column-parallel),
full attention for those heads, and a partial output projection
(row-parallel).  The host sums the 4 partials per batch and adds bo.
"""

import math

import numpy as np

import ml_dtypes
import concourse.bass as bass
import concourse.bacc as bacc
import concourse.tile as tile
from concourse import mybir
from concourse.bass_utils import run_bass_kernel_spmd

F32 = mybir.dt.float32
F32R = mybir.dt.float32r
AF = mybir.ActivationFunctionType

B = 2
S = 2048
D = 2048
DK = 128
HPC = 4          # heads per core
ND = D // 128    # 16 d_model tiles
NKT = S // 128   # 16 k tiles
NSC = S // 512   # 4 s chunks of 512
NQP = S // 512   # 4 q passes of 512
NQT = S // 128   # 16 q tiles
NNC = D // 512   # 4 n chunks of 512
SCALE = 1.0 / math.sqrt(DK)
PROJ_BF16 = True
BF16 = mybir.dt.bfloat16
PDT = BF16 if PROJ_BF16 else F32R
SC_BUFS = 3
PA_BUFS = 2
DEN_BUFS = 1
XUN_BUFS = 2
XN_BUFS = 6
REPS = 1
EXP_BUFS = 4

# toggled by test.py for profiling
TRACE = False
LAST_EXEC_NS = None

_NC_CACHE = None


def _bcast_ap(ap, parts):
    """[1, N] AP -> [parts, N] AP with zero partition step (DRAM src only)."""
    return bass.AP(tensor=ap.tensor, offset=ap.offset, ap=[[0, parts]] + list(ap.ap[1:]))


def build_nc():
    from contextlib import ExitStack

    nc = bacc.Bacc("TRN2", target_bir_lowering=False, debug=False)

    xqT = nc.dram_tensor("xqT", (D, S), PDT, kind="ExternalInput")[:]
    xkT = nc.dram_tensor("xkT", (D, S), PDT, kind="ExternalInput")[:]
    xvT = nc.dram_tensor("xvT", (D, S), PDT, kind="ExternalInput")[:]
    wq = nc.dram_tensor("wq", (D, 512), PDT, kind="ExternalInput")[:]
    wk = nc.dram_tensor("wk", (D, 512), PDT, kind="ExternalInput")[:]
    wv = nc.dram_tensor("wv", (D, 512), PDT, kind="ExternalInput")[:]
    wo = nc.dram_tensor("wo", (512, D), F32R, kind="ExternalInput")[:]
    bqT = nc.dram_tensor("bqT", (128, HPC), F32, kind="ExternalInput")[:]
    bkT = nc.dram_tensor("bkT", (128, HPC), F32, kind="ExternalInput")[:]
    bvb = nc.dram_tensor("bvb", (128, 512), F32, kind="ExternalInput")[:]
    cosT = nc.dram_tensor("cosT", (128, S), F32R, kind="ExternalInput")[:]
    sinTs = nc.dram_tensor("sinTs", (128, S), F32R, kind="ExternalInput")[:]
    ones = nc.dram_tensor("ones", (128, 1), F32R, kind="ExternalInput")[:]
    out = nc.dram_tensor("out", (S, D), F32, kind="ExternalOutput")[:]

    with tile.TileContext(nc) as tc, ExitStack() as ctx:
        consts = ctx.enter_context(tc.tile_pool(name="consts", bufs=1))
        wpool = ctx.enter_context(tc.tile_pool(name="wpool", bufs=20))
        xpool = ctx.enter_context(tc.tile_pool(name="xpool", bufs=22))
        big = ctx.enter_context(tc.tile_pool(name="big", bufs=4))
        kv = ctx.enter_context(tc.tile_pool(name="kv", bufs=2))
        work = ctx.enter_context(tc.tile_pool(name="work", bufs=3))
        psum = ctx.enter_context(tc.tile_pool(name="psum", bufs=2, space="PSUM"))
        dram = ctx.enter_context(tc.tile_pool(name="dram", bufs=1, space="DRAM"))

        bq_sb = consts.tile([128, HPC], F32, tag="bq", name="bq")
        nc.sync.dma_start(bq_sb, bqT)
        bk_sb = consts.tile([128, HPC], F32, tag="bk", name="bk")
        nc.sync.dma_start(bk_sb, bkT)
        bvb_sb = consts.tile([128, 512], F32, tag="bvb", name="bvb")
        ones_sb = consts.tile([128, 1], F32R, tag="ones", name="ones")
        cos_sb = kv.tile([128, S], F32R, tag="ktS", name="ktS")
        sin_sb = kv.tile([128, S], F32R, tag="vtS", name="vtS")

        # dram scratch
        ktR = [dram.tile([128, S], F32R, tag=f"ktR{h}", name=f"ktR{h}")
               for h in range(HPC)]
        vR = [dram.tile([128, S], F32R, tag=f"vR{h}", name=f"vR{h}") for h in range(HPC)]
        xnR = [[dram.tile([128, 512], F32R, tag=f"xnR{h}_{p}", name=f"xnR{h}_{p}")
                for p in range(NQP)] for h in range(HPC)]
        recD = [[dram.tile([1, 512], F32, tag=f"recD{h}_{p}", name=f"recD{h}_{p}")
                 for p in range(NQP)] for h in range(HPC)]

        for _rep in range(REPS):
            QT = [big.tile([128, S], F32R, tag="qt", name="qt") for _ in range(HPC)]

            # ---------------- Phase A: projections + RoPE ----------------
            def rope_pair(tgt, cp):
                """tgt [128,1024] (chunk pair cp): tgt = tgt*cos + swap(tgt)*sin'."""
                sl = bass.ts(cp, 1024)
                tmp = work.tile([128, 1024], F32R, tag="ropetmp", bufs=2, name="ropetmp")
                nc.gpsimd.dma_start(tmp[0:64, :], tgt[64:128, :])
                nc.gpsimd.dma_start(tmp[64:128, :], tgt[0:64, :])
                nc.vector.tensor_mul(tmp, tmp, sin_sb[:, sl])
                nc.vector.tensor_mul(tgt, tgt, cos_sb[:, sl])
                nc.vector.tensor_add(tgt, tgt, tmp)

            first = True
            for which, xT_d, w_d, b_sb in (("q", xqT, wq, bq_sb), ("k", xkT, wk, bk_sb)):
                wt = []
                for cp in range(2):
                    xt = []
                    for d in range(ND):
                        if cp == 0:
                            t = wpool.tile([128, 512], PDT, tag="w", name="w")
                            nc.gpsimd.dma_start(t, w_d[d * 128:(d + 1) * 128, :])
                            wt.append(t)
                        t = xpool.tile([128, 1024], PDT, tag="x", name="x")
                        nc.sync.dma_start(
                            t, xT_d[d * 128:(d + 1) * 128, cp * 1024:(cp + 1) * 1024])
                        xt.append(t)
                    if first:
                        nc.gpsimd.dma_start(cos_sb, cosT)
                        nc.gpsimd.dma_start(sin_sb, sinTs)
                        nc.gpsimd.dma_start(bvb_sb, bvb)
                        nc.gpsimd.dma_start(ones_sb, ones)
                        first = False
                    for h in range(HPC):
                        if which == "q":
                            tgt = QT[h][:, cp * 1024:(cp + 1) * 1024]
                        else:
                            tgt = work.tile([128, 1024], F32R, tag="kstage", bufs=2, name="kstage")
                        for half in range(2):
                            ps = psum.tile([128, 512], F32, tag="pa", bufs=PA_BUFS, name="pa")
                            for d in range(ND):
                                nc.tensor.matmul(
                                    ps,
                                    lhsT=wt[d][:, h * 128:(h + 1) * 128],
                                    rhs=xt[d][:, half * 512:(half + 1) * 512],
                                    start=(d == 0), stop=(d == ND - 1),
                                )
                            nc.scalar.activation(
                                tgt[:, half * 512:(half + 1) * 512], ps,
                                AF.Identity, bias=b_sb[:, h:h + 1])
                        rope_pair(tgt, cp)
                        if which == "k":
                            nc.gpsimd.dma_start(
                                ktR[h][:, cp * 1024:(cp + 1) * 1024], tgt)

            # V projection (natural [s, head*dk] layout)
            wvt = []
            for cp in range(2):
                xt = []
                for d in range(ND):
                    if cp == 0:
                        t = wpool.tile([128, 512], PDT, tag="w", name="w")
                        nc.gpsimd.dma_start(t, wv[d * 128:(d + 1) * 128, :])
                        wvt.append(t)
                    t = xpool.tile([128, 1024], PDT, tag="x", name="x")
                    nc.sync.dma_start(
                        t, xvT[d * 128:(d + 1) * 128, cp * 1024:(cp + 1) * 1024])
                    xt.append(t)
                for sti in range(8):
                    st = cp * 8 + sti
                    ps = psum.tile([128, 512], F32, tag="pa", bufs=PA_BUFS, name="pa")
                    for d in range(ND):
                        nc.tensor.matmul(
                            ps,
                            lhsT=xt[d][:, sti * 128:(sti + 1) * 128],
                            rhs=wvt[d][:],
                            start=(d == 0), stop=(d == ND - 1),
                        )
                    vstage = work.tile([128, 512], F32R, tag="vstage", bufs=2, name="vstage")
                    nc.vector.tensor_add(vstage, ps, bvb_sb)
                    for h in range(HPC):
                        nc.gpsimd.dma_start(
                            vR[h][:, st * 128:(st + 1) * 128],
                            vstage[:, h * 128:(h + 1) * 128])

            # ---------------- Phase B: attention per head ----------------
            for h in range(HPC):
                ktS = kv.tile([128, S], F32R, tag="ktS", name="ktS")
                nc.sync.dma_start(ktS, ktR[h][:, :])
                vtS = kv.tile([128, S], F32R, tag="vtS", name="vtS")
                # gather head h columns from the st-major vR in one strided DMA
                nc.sync.dma_start(vtS, vR[h][:, :])
                for p in range(NQP):
                    qsl = bass.ts(p, 512)
                    ps_x = psum.tile([128, 512], F32, tag="xun", bufs=XUN_BUFS, name="xun")
                    ps_d = psum.tile([1, 512], F32, tag="den", bufs=DEN_BUFS, name="den")
                    for kt in range(NKT):
                        ps_s = psum.tile([128, 512], F32, tag="sc", bufs=SC_BUFS, name="sc")
                        nc.tensor.matmul(
                            ps_s,
                            lhsT=ktS[:, kt * 128:(kt + 1) * 128],
                            rhs=QT[h][:, qsl],
                            start=True, stop=True,
                        )
                        ex = work.tile([128, 512], F32R, tag="expT", bufs=EXP_BUFS, name="expT")
                        nc.scalar.activation(ex, ps_s, AF.Exp, scale=SCALE)
                        nc.tensor.matmul(
                            ps_x,
                            lhsT=vtS[:, kt * 128:(kt + 1) * 128],
                            rhs=ex[:],
                            start=(kt == 0), stop=(kt == NKT - 1),
                        )
                        nc.tensor.matmul(
                            ps_d,
                            lhsT=ones_sb[:],
                            rhs=ex[:],
                            start=(kt == 0), stop=(kt == NKT - 1),
                        )
                    # softmax denominator: reciprocal + broadcast across partitions
                    denR = work.tile([1, 512], F32, tag="denR", bufs=1, name="denR")
                    nc.scalar.copy(denR, ps_d)
                    nc.vector.reciprocal(denR, denR)
                    nc.gpsimd.dma_start(recD[h][p][:], denR)
                    recB = work.tile([128, 512], F32, tag="recB", bufs=1, name="recB")
                    nc.gpsimd.dma_start(recB, _bcast_ap(recD[h][p][:], 128))
                    xn = work.tile([128, 512], F32R, tag="xn", bufs=2, name="xn")
                    nc.vector.tensor_mul(xn, ps_x, recB)
                    nc.sync.dma_start(xnR[h][p][:, :], xn)

            # ---------------- Phase C: output projection ----------------
            wot = []
            for h in range(HPC):
                for ncl in range(NNC):
                    t = wpool.tile([128, 512], F32R, tag="w", name="w")
                    nc.gpsimd.dma_start(
                        t, wo[h * 128:(h + 1) * 128, ncl * 512:(ncl + 1) * 512])
                    wot.append(t)
            for qc in range(4):
                xnt = []
                for h in range(HPC):
                    t = xpool.tile([128, 512], F32R, tag="xn_in", bufs=XN_BUFS, name="xn_in")
                    nc.sync.dma_start(t, xnR[h][qc][:, :])
                    xnt.append(t)
                for qi in range(4):
                    qt = qc * 4 + qi
                    orow = [work.tile([128, 1024], F32, tag=f"orow{i}", bufs=1, name=f"orow{i}")
                            for i in range(2)]
                    for ncl in range(NNC):
                        ps = psum.tile([128, 512], F32, tag="pa", bufs=PA_BUFS, name="pa")
                        for h in range(HPC):
                            nc.tensor.matmul(
                                ps,
                                lhsT=xnt[h][:, qi * 128:(qi + 1) * 128],
                                rhs=wot[h * NNC + ncl][:],
                                start=(h == 0), stop=(h == HPC - 1),
                            )
                        nc.vector.tensor_copy(
                            orow[ncl // 2][:, (ncl % 2) * 512:(ncl % 2 + 1) * 512], ps)
                    for half in range(2):
                        nc.sync.dma_start(
                            out[qt * 128:(qt + 1) * 128, half * 1024:(half + 1) * 1024],
                            orow[half])

    nc.finalize()
    return nc


def _rope_tables():
    inv_freq = (1.0 / (10000.0 ** (np.arange(0, DK, 2, dtype=np.float32) / DK))).astype(np.float32)
    pos = np.arange(S, dtype=np.float32)
    freqs = pos[:, None] * inv_freq[None, :]
    emb = np.concatenate([freqs, freqs], axis=-1)  # [S, DK]
    cos = np.cos(emb).astype(np.float32)
    sin = np.sin(emb).astype(np.float32)
    cosT = np.ascontiguousarray(cos.T)  # [DK, S]
    sinT = np.ascontiguousarray(sin.T)
    sinTs = sinT.copy()
    sinTs[0:64] *= -1.0
    return cosT, sinTs


def kernel(query, key, value, Wq, bq, Wk, bk, Wv, bv, Wo, bo):
    global _NC_CACHE, LAST_EXEC_NS
    query = np.asarray(query, dtype=np.float32)
    key = np.asarray(key, dtype=np.float32)
    value = np.asarray(value, dtype=np.float32)
    Wq = np.asarray(Wq, dtype=np.float32)
    Wk = np.asarray(Wk, dtype=np.float32)
    Wv = np.asarray(Wv, dtype=np.float32)
    Wo = np.asarray(Wo, dtype=np.float32)
    bq = np.asarray(bq, dtype=np.float32)
    bk = np.asarray(bk, dtype=np.float32)
    bv = np.asarray(bv, dtype=np.float32)
    bo = np.asarray(bo, dtype=np.float32)

    if _NC_CACHE is None:
        _NC_CACHE = build_nc()
    nc = _NC_CACHE

    cosT, sinTs = _rope_tables()
    WqT = np.ascontiguousarray(Wq.T)
    WkT = np.ascontiguousarray(Wk.T)
    WvT = np.ascontiguousarray(Wv.T)
    WoT = np.ascontiguousarray(Wo.T)
    pdt = ml_dtypes.bfloat16 if PROJ_BF16 else np.float32
    xT = {}
    for b in range(B):
        xT[("q", b)] = np.ascontiguousarray(query[b].T).astype(pdt)
        xT[("k", b)] = np.ascontiguousarray(key[b].T).astype(pdt)
        xT[("v", b)] = np.ascontiguousarray(value[b].T).astype(pdt)

    in_maps = []
    for c in range(8):
        b = c // 4
        g = c % 4
        cols = slice(g * 512, (g + 1) * 512)
        in_maps.append({
            "xqT": xT[("q", b)],
            "xkT": xT[("k", b)],
            "xvT": xT[("v", b)],
            "wq": np.ascontiguousarray(WqT[:, cols]).astype(pdt),
            "wk": np.ascontiguousarray(WkT[:, cols]).astype(pdt),
            "wv": np.ascontiguousarray(WvT[:, cols]).astype(pdt),
            "wo": np.ascontiguousarray(WoT[cols, :]),
            "bqT": np.ascontiguousarray(bq[cols].reshape(HPC, 128).T),
            "bkT": np.ascontiguousarray(bk[cols].reshape(HPC, 128).T),
            "bvb": np.ascontiguousarray(
                np.broadcast_to(bv[cols][None, :], (128, 512))),
            "cosT": cosT,
            "sinTs": sinTs,
            "ones": np.ones((128, 1), dtype=np.float32),
        })

    res = run_bass_kernel_spmd(nc, in_maps, core_ids=list(range(8)))
    LAST_EXEC_NS = res.exec_time_ns

    parts = [res.results[c]["out"] for c in range(8)]
    outp = np.empty((B, S, D), dtype=np.float32)
    for b in range(B):
        acc = parts[4 * b].astype(np.float32)
        for g in range(1, 4):
            acc = acc + parts[4 * b + g]
        outp[b] = acc + bo[None, :]
    return outp



# revision 25
# speedup vs baseline: 1.1151x; 1.1151x over previous
"""Multi-head attention (16 heads, RoPE, B=2, S=2048, D=2048) on 8 trn2 cores.

Sharding: core c handles batch b=c//4 and heads [4g, 4g+4) with g=c%4.
Each core computes Q/K/V projections for its 4 heads (column-parallel),
full attention for those heads, and a partial output projection
(row-parallel).  The host sums the 4 partials per batch and adds bo.
"""

import math

import numpy as np

import ml_dtypes
import concourse.bass as bass
import concourse.bacc as bacc
import concourse.tile as tile
from concourse import mybir
from concourse.bass_utils import run_bass_kernel_spmd

F32 = mybir.dt.float32
F32R = mybir.dt.float32r
AF = mybir.ActivationFunctionType

B = 2
S = 2048
D = 2048
DK = 128
HPC = 4          # heads per core
ND = D // 128    # 16 d_model tiles
NKT = S // 128   # 16 k tiles
NSC = S // 512   # 4 s chunks of 512
NQP = S // 512   # 4 q passes of 512
NQT = S // 128   # 16 q tiles
NNC = D // 512   # 4 n chunks of 512
SCALE = 1.0 / math.sqrt(DK)
PROJ_BF16 = True
BF16 = mybir.dt.bfloat16
PDT = BF16 if PROJ_BF16 else F32R
SC_BUFS = 3
PA_BUFS = 2
DEN_BUFS = 1
XUN_BUFS = 2
XN_BUFS = 6
REPS = 1
EXP_BUFS = 4

# exec_time_ns from the runner when NTFF profiling is available (None here)
LAST_EXEC_NS = None

_NC_CACHE = None


def _bcast_ap(ap, parts):
    """[1, N] AP -> [parts, N] AP with zero partition step (DRAM src only)."""
    return bass.AP(tensor=ap.tensor, offset=ap.offset, ap=[[0, parts]] + list(ap.ap[1:]))


def build_nc():
    from contextlib import ExitStack

    nc = bacc.Bacc("TRN2", target_bir_lowering=False, debug=False)

    xqT = nc.dram_tensor("xqT", (D, S), PDT, kind="ExternalInput")[:]
    xkT = nc.dram_tensor("xkT", (D, S), PDT, kind="ExternalInput")[:]
    xvT = nc.dram_tensor("xvT", (D, S), PDT, kind="ExternalInput")[:]
    wq = nc.dram_tensor("wq", (D, 512), PDT, kind="ExternalInput")[:]
    wk = nc.dram_tensor("wk", (D, 512), PDT, kind="ExternalInput")[:]
    wv = nc.dram_tensor("wv", (D, 512), PDT, kind="ExternalInput")[:]
    wo = nc.dram_tensor("wo", (512, D), F32R, kind="ExternalInput")[:]
    bqT = nc.dram_tensor("bqT", (128, HPC), F32, kind="ExternalInput")[:]
    bkT = nc.dram_tensor("bkT", (128, HPC), F32, kind="ExternalInput")[:]
    bvb = nc.dram_tensor("bvb", (128, 512), F32, kind="ExternalInput")[:]
    cosT = nc.dram_tensor("cosT", (128, S), F32R, kind="ExternalInput")[:]
    sinTs = nc.dram_tensor("sinTs", (128, S), F32R, kind="ExternalInput")[:]
    ones = nc.dram_tensor("ones", (128, 1), F32R, kind="ExternalInput")[:]
    out = nc.dram_tensor("out", (S, D), F32, kind="ExternalOutput")[:]

    with tile.TileContext(nc) as tc, ExitStack() as ctx:
        consts = ctx.enter_context(tc.tile_pool(name="consts", bufs=1))
        wpool = ctx.enter_context(tc.tile_pool(name="wpool", bufs=20))
        xpool = ctx.enter_context(tc.tile_pool(name="xpool", bufs=22))
        big = ctx.enter_context(tc.tile_pool(name="big", bufs=4))
        kv = ctx.enter_context(tc.tile_pool(name="kv", bufs=2))
        work = ctx.enter_context(tc.tile_pool(name="work", bufs=3))
        psum = ctx.enter_context(tc.tile_pool(name="psum", bufs=2, space="PSUM"))
        dram = ctx.enter_context(tc.tile_pool(name="dram", bufs=1, space="DRAM"))

        bq_sb = consts.tile([128, HPC], F32, tag="bq", name="bq")
        nc.sync.dma_start(bq_sb, bqT)
        bk_sb = consts.tile([128, HPC], F32, tag="bk", name="bk")
        nc.sync.dma_start(bk_sb, bkT)
        bvb_sb = consts.tile([128, 512], F32, tag="bvb", name="bvb")
        ones_sb = consts.tile([128, 1], F32R, tag="ones", name="ones")
        cos_sb = kv.tile([128, S], F32R, tag="ktS", name="ktS")
        sin_sb = kv.tile([128, S], F32R, tag="vtS", name="vtS")

        # dram scratch
        ktR = [dram.tile([128, S], F32R, tag=f"ktR{h}", name=f"ktR{h}")
               for h in range(HPC)]
        vR = dram.tile([NKT, 128, 512], F32R, tag="vR", name="vR")
        xnR = [[dram.tile([128, 512], F32R, tag=f"xnR{h}_{p}", name=f"xnR{h}_{p}")
                for p in range(NQP)] for h in range(HPC)]
        recD = [[dram.tile([1, 512], F32, tag=f"recD{h}_{p}", name=f"recD{h}_{p}")
                 for p in range(NQP)] for h in range(HPC)]

        for _rep in range(REPS):
            QT = [big.tile([128, S], F32R, tag="qt", name="qt") for _ in range(HPC)]

            # ---------------- Phase A: projections + RoPE ----------------
            def rope_pair(tgt, cp):
                """tgt [128,1024] (chunk pair cp): tgt = tgt*cos + swap(tgt)*sin'."""
                sl = bass.ts(cp, 1024)
                tmp = work.tile([128, 1024], F32R, tag="ropetmp", bufs=2, name="ropetmp")
                nc.gpsimd.dma_start(tmp[0:64, :], tgt[64:128, :])
                nc.gpsimd.dma_start(tmp[64:128, :], tgt[0:64, :])
                nc.vector.tensor_mul(tmp, tmp, sin_sb[:, sl])
                nc.vector.tensor_mul(tgt, tgt, cos_sb[:, sl])
                nc.vector.tensor_add(tgt, tgt, tmp)

            first = True
            for which, xT_d, w_d, b_sb in (("q", xqT, wq, bq_sb), ("k", xkT, wk, bk_sb)):
                wt = []
                for cp in range(2):
                    xt = []
                    for d in range(ND):
                        if cp == 0:
                            t = wpool.tile([128, 512], PDT, tag="w", name="w")
                            nc.sync.dma_start(t, w_d[d * 128:(d + 1) * 128, :])
                            wt.append(t)
                        t = xpool.tile([128, 1024], PDT, tag="x", name="x")
                        nc.sync.dma_start(
                            t, xT_d[d * 128:(d + 1) * 128, cp * 1024:(cp + 1) * 1024])
                        xt.append(t)
                    if first:
                        nc.gpsimd.dma_start(cos_sb, cosT)
                        nc.gpsimd.dma_start(sin_sb, sinTs)
                        nc.gpsimd.dma_start(bvb_sb, bvb)
                        nc.gpsimd.dma_start(ones_sb, ones)
                        first = False
                    for h in range(HPC):
                        if which == "q":
                            tgt = QT[h][:, cp * 1024:(cp + 1) * 1024]
                        else:
                            tgt = work.tile([128, 1024], F32R, tag="kstage", bufs=4, name="kstage")
                        for half in range(2):
                            ps = psum.tile([128, 512], F32, tag="pa", bufs=PA_BUFS, name="pa")
                            for d in range(ND):
                                nc.tensor.matmul(
                                    ps,
                                    lhsT=wt[d][:, h * 128:(h + 1) * 128],
                                    rhs=xt[d][:, half * 512:(half + 1) * 512],
                                    start=(d == 0), stop=(d == ND - 1),
                                )
                            nc.scalar.activation(
                                tgt[:, half * 512:(half + 1) * 512], ps,
                                AF.Identity, bias=b_sb[:, h:h + 1])
                        rope_pair(tgt, cp)
                        if which == "k":
                            nc.gpsimd.dma_start(
                                ktR[h][:, cp * 1024:(cp + 1) * 1024], tgt)

            # V projection (natural [s, head*dk] layout)
            wvt = []
            for cp in range(2):
                xt = []
                for d in range(ND):
                    if cp == 0:
                        t = wpool.tile([128, 512], PDT, tag="w", name="w")
                        nc.sync.dma_start(t, wv[d * 128:(d + 1) * 128, :])
                        wvt.append(t)
                    t = xpool.tile([128, 1024], PDT, tag="x", name="x")
                    nc.sync.dma_start(
                        t, xvT[d * 128:(d + 1) * 128, cp * 1024:(cp + 1) * 1024])
                    xt.append(t)
                for sti in range(8):
                    st = cp * 8 + sti
                    ps = psum.tile([128, 512], F32, tag="pa", bufs=PA_BUFS, name="pa")
                    for d in range(ND):
                        nc.tensor.matmul(
                            ps,
                            lhsT=xt[d][:, sti * 128:(sti + 1) * 128],
                            rhs=wvt[d][:],
                            start=(d == 0), stop=(d == ND - 1),
                        )
                    vstage = work.tile([128, 512], F32R, tag="vstage", bufs=2, name="vstage")
                    nc.vector.tensor_add(vstage, ps, bvb_sb)
                    nc.sync.dma_start(vR[st, :, :], vstage)

            # ---------------- Phase B: attention per head ----------------
            for h in range(HPC):
                ktS = kv.tile([128, S], F32R, tag="ktS", name="ktS")
                for kh in range(2):
                    nc.sync.dma_start(
                        ktS[:, kh * 1024:(kh + 1) * 1024],
                        ktR[h][:, kh * 1024:(kh + 1) * 1024])
                vtS = kv.tile([128, S], F32R, tag="vtS", name="vtS")
                # gather head h columns from the st-major vR in quarter chunks
                for vq in range(4):
                    nc.sync.dma_start(
                        vtS[:, vq * 512:(vq + 1) * 512].rearrange(
                            "p (t c) -> p t c", t=4),
                        vR[vq * 4:(vq + 1) * 4, :, h * 128:(h + 1) * 128]
                        .rearrange("t p c -> p t c"))
                for p in range(NQP):
                    qsl = bass.ts(p, 512)
                    ps_x = psum.tile([128, 512], F32, tag="xun", bufs=XUN_BUFS, name="xun")
                    ps_d = psum.tile([1, 512], F32, tag="den", bufs=DEN_BUFS, name="den")
                    for kt in range(NKT):
                        ps_s = psum.tile([128, 512], F32, tag="sc", bufs=SC_BUFS, name="sc")
                        nc.tensor.matmul(
                            ps_s,
                            lhsT=ktS[:, kt * 128:(kt + 1) * 128],
                            rhs=QT[h][:, qsl],
                            start=True, stop=True,
                        )
                        ex = work.tile([128, 512], F32R, tag="expT", bufs=EXP_BUFS, name="expT")
                        nc.scalar.activation(ex, ps_s, AF.Exp, scale=SCALE)
                        nc.tensor.matmul(
                            ps_x,
                            lhsT=vtS[:, kt * 128:(kt + 1) * 128],
                            rhs=ex[:],
                            start=(kt == 0), stop=(kt == NKT - 1),
                        )
                        nc.tensor.matmul(
                            ps_d,
                            lhsT=ones_sb[:],
                            rhs=ex[:],
                            start=(kt == 0), stop=(kt == NKT - 1),
                        )
                    # softmax denominator: reciprocal + broadcast across partitions
                    denR = work.tile([1, 512], F32, tag="denR", bufs=1, name="denR")
                    nc.scalar.copy(denR, ps_d)
                    nc.vector.reciprocal(denR, denR)
                    nc.gpsimd.dma_start(recD[h][p][:], denR)
                    recB = work.tile([128, 512], F32, tag="recB", bufs=1, name="recB")
                    nc.gpsimd.dma_start(recB, _bcast_ap(recD[h][p][:], 128))
                    xn = work.tile([128, 512], F32R, tag="xn", bufs=2, name="xn")
                    nc.vector.tensor_mul(xn, ps_x, recB)
                    nc.sync.dma_start(xnR[h][p][:, :], xn)

            # ---------------- Phase C: output projection ----------------
            wot = []
            for h in range(HPC):
                for ncl in range(NNC):
                    t = wpool.tile([128, 512], F32R, tag="w", name="w")
                    nc.sync.dma_start(
                        t, wo[h * 128:(h + 1) * 128, ncl * 512:(ncl + 1) * 512])
                    wot.append(t)
            for qc in range(4):
                xnt = []
                for h in range(HPC):
                    t = xpool.tile([128, 512], F32R, tag="xn_in", bufs=XN_BUFS, name="xn_in")
                    nc.sync.dma_start(t, xnR[h][qc][:, :])
                    xnt.append(t)
                for qi in range(4):
                    qt = qc * 4 + qi
                    orow = [work.tile([128, 1024], F32, tag="kstage", bufs=4, name=f"orow{i}")
                            for i in range(2)]
                    for ncl in range(NNC):
                        _tg, _tb = ("pa", PA_BUFS) if ncl % 2 == 0 else ("sc", SC_BUFS)
                        ps = psum.tile([128, 512], F32, tag=_tg, bufs=_tb, name="pa")
                        for h in range(HPC):
                            nc.tensor.matmul(
                                ps,
                                lhsT=xnt[h][:, qi * 128:(qi + 1) * 128],
                                rhs=wot[h * NNC + ncl][:],
                                start=(h == 0), stop=(h == HPC - 1),
                            )
                        nc.vector.tensor_copy(
                            orow[ncl // 2][:, (ncl % 2) * 512:(ncl % 2 + 1) * 512], ps)
                    for half in range(2):
                        nc.sync.dma_start(
                            out[qt * 128:(qt + 1) * 128, half * 1024:(half + 1) * 1024],
                            orow[half])

    nc.finalize()
    return nc


def _rope_tables():
    inv_freq = (1.0 / (10000.0 ** (np.arange(0, DK, 2, dtype=np.float32) / DK))).astype(np.float32)
    pos = np.arange(S, dtype=np.float32)
    freqs = pos[:, None] * inv_freq[None, :]
    emb = np.concatenate([freqs, freqs], axis=-1)  # [S, DK]
    cos = np.cos(emb).astype(np.float32)
    sin = np.sin(emb).astype(np.float32)
    cosT = np.ascontiguousarray(cos.T)  # [DK, S]
    sinT = np.ascontiguousarray(sin.T)
    sinTs = sinT.copy()
    sinTs[0:64] *= -1.0
    return cosT, sinTs


def kernel(query, key, value, Wq, bq, Wk, bk, Wv, bv, Wo, bo):
    global _NC_CACHE, LAST_EXEC_NS
    query = np.asarray(query, dtype=np.float32)
    key = np.asarray(key, dtype=np.float32)
    value = np.asarray(value, dtype=np.float32)
    Wq = np.asarray(Wq, dtype=np.float32)
    Wk = np.asarray(Wk, dtype=np.float32)
    Wv = np.asarray(Wv, dtype=np.float32)
    Wo = np.asarray(Wo, dtype=np.float32)
    bq = np.asarray(bq, dtype=np.float32)
    bk = np.asarray(bk, dtype=np.float32)
    bv = np.asarray(bv, dtype=np.float32)
    bo = np.asarray(bo, dtype=np.float32)

    if _NC_CACHE is None:
        _NC_CACHE = build_nc()
    nc = _NC_CACHE

    cosT, sinTs = _rope_tables()
    WqT = np.ascontiguousarray(Wq.T)
    WkT = np.ascontiguousarray(Wk.T)
    WvT = np.ascontiguousarray(Wv.T)
    WoT = np.ascontiguousarray(Wo.T)
    pdt = ml_dtypes.bfloat16 if PROJ_BF16 else np.float32
    xT = {}
    for b in range(B):
        xT[("q", b)] = np.ascontiguousarray(query[b].T).astype(pdt)
        xT[("k", b)] = np.ascontiguousarray(key[b].T).astype(pdt)
        xT[("v", b)] = np.ascontiguousarray(value[b].T).astype(pdt)

    in_maps = []
    for c in range(8):
        b = c // 4
        g = c % 4
        cols = slice(g * 512, (g + 1) * 512)
        in_maps.append({
            "xqT": xT[("q", b)],
            "xkT": xT[("k", b)],
            "xvT": xT[("v", b)],
            "wq": np.ascontiguousarray(WqT[:, cols]).astype(pdt),
            "wk": np.ascontiguousarray(WkT[:, cols]).astype(pdt),
            "wv": np.ascontiguousarray(WvT[:, cols]).astype(pdt),
            "wo": np.ascontiguousarray(WoT[cols, :]),
            "bqT": np.ascontiguousarray(bq[cols].reshape(HPC, 128).T),
            "bkT": np.ascontiguousarray(bk[cols].reshape(HPC, 128).T),
            "bvb": np.ascontiguousarray(
                np.broadcast_to(bv[cols][None, :], (128, 512))),
            "cosT": cosT,
            "sinTs": sinTs,
            "ones": np.ones((128, 1), dtype=np.float32),
        })

    res = run_bass_kernel_spmd(nc, in_maps, core_ids=list(range(8)))
    LAST_EXEC_NS = res.exec_time_ns

    parts = [res.results[c]["out"] for c in range(8)]
    outp = np.empty((B, S, D), dtype=np.float32)
    for b in range(B):
        acc = parts[4 * b].astype(np.float32)
        for g in range(1, 4):
            acc = acc + parts[4 * b + g]
        outp[b] = acc + bo[None, :]
    return outp



# revision 28
# speedup vs baseline: 1.1179x; 1.0025x over previous
"""Multi-head attention (16 heads, RoPE, B=2, S=2048, D=2048) on 8 trn2 cores.

Sharding: core c handles batch b=c//4 and heads [4g, 4g+4) with g=c%4.
Each core computes Q/K/V projections for its 4 heads (column-parallel),
full attention for those heads, and a partial output projection
(row-parallel).  The host sums the 4 partials per batch and adds bo.
"""

import math

import numpy as np

import ml_dtypes
import concourse.bass as bass
import concourse.bacc as bacc
import concourse.tile as tile
from concourse import mybir
from concourse.bass_utils import run_bass_kernel_spmd

F32 = mybir.dt.float32
F32R = mybir.dt.float32r
AF = mybir.ActivationFunctionType

B = 2
S = 2048
D = 2048
DK = 128
HPC = 4          # heads per core
ND = D // 128    # 16 d_model tiles
NKT = S // 128   # 16 k tiles
NSC = S // 512   # 4 s chunks of 512
NQP = S // 512   # 4 q passes of 512
NQT = S // 128   # 16 q tiles
NNC = D // 512   # 4 n chunks of 512
SCALE = 1.0 / math.sqrt(DK)
PROJ_BF16 = True
BF16 = mybir.dt.bfloat16
PDT = BF16 if PROJ_BF16 else F32R
SC_BUFS = 3
PA_BUFS = 2
DEN_BUFS = 1
XUN_BUFS = 2
XN_BUFS = 6
REPS = 1
EXP_BUFS = 4

# exec_time_ns from the runner when NTFF profiling is available (None here)
LAST_EXEC_NS = None

_NC_CACHE = None


def _bcast_ap(ap, parts):
    """[1, N] AP -> [parts, N] AP with zero partition step (DRAM src only)."""
    return bass.AP(tensor=ap.tensor, offset=ap.offset, ap=[[0, parts]] + list(ap.ap[1:]))


def build_nc():
    from contextlib import ExitStack

    nc = bacc.Bacc("TRN2", target_bir_lowering=False, debug=False)

    xqT = nc.dram_tensor("xqT", (D, S), PDT, kind="ExternalInput")[:]
    xkT = nc.dram_tensor("xkT", (D, S), PDT, kind="ExternalInput")[:]
    xvT = nc.dram_tensor("xvT", (D, S), PDT, kind="ExternalInput")[:]
    wq = nc.dram_tensor("wq", (D, 512), PDT, kind="ExternalInput")[:]
    wk = nc.dram_tensor("wk", (D, 512), PDT, kind="ExternalInput")[:]
    wv = nc.dram_tensor("wv", (D, 512), PDT, kind="ExternalInput")[:]
    wo = nc.dram_tensor("wo", (512, D), F32R, kind="ExternalInput")[:]
    bqT = nc.dram_tensor("bqT", (128, HPC), F32, kind="ExternalInput")[:]
    bkT = nc.dram_tensor("bkT", (128, HPC), F32, kind="ExternalInput")[:]
    bvb = nc.dram_tensor("bvb", (128, 512), F32, kind="ExternalInput")[:]
    cosT = nc.dram_tensor("cosT", (128, S), F32R, kind="ExternalInput")[:]
    sinTs = nc.dram_tensor("sinTs", (128, S), F32R, kind="ExternalInput")[:]
    ones = nc.dram_tensor("ones", (128, 1), F32R, kind="ExternalInput")[:]
    out = nc.dram_tensor("out", (S, D), F32, kind="ExternalOutput")[:]

    with tile.TileContext(nc) as tc, ExitStack() as ctx:
        consts = ctx.enter_context(tc.tile_pool(name="consts", bufs=1))
        wpool = ctx.enter_context(tc.tile_pool(name="wpool", bufs=20))
        xpool = ctx.enter_context(tc.tile_pool(name="xpool", bufs=22))
        big = ctx.enter_context(tc.tile_pool(name="big", bufs=4))
        kv = ctx.enter_context(tc.tile_pool(name="kv", bufs=2))
        work = ctx.enter_context(tc.tile_pool(name="work", bufs=3))
        psum = ctx.enter_context(tc.tile_pool(name="psum", bufs=2, space="PSUM"))
        dram = ctx.enter_context(tc.tile_pool(name="dram", bufs=1, space="DRAM"))

        bq_sb = consts.tile([128, HPC], F32, tag="bq", name="bq")
        nc.sync.dma_start(bq_sb, bqT)
        bk_sb = consts.tile([128, HPC], F32, tag="bk", name="bk")
        nc.sync.dma_start(bk_sb, bkT)
        bvb_sb = consts.tile([128, 512], F32, tag="bvb", name="bvb")
        ones_sb = consts.tile([128, 1], F32R, tag="ones", name="ones")
        cos_sb = kv.tile([128, S], F32R, tag="ktS", name="ktS")
        sin_sb = kv.tile([128, S], F32R, tag="vtS", name="vtS")

        # dram scratch
        ktR = [dram.tile([128, S], F32R, tag=f"ktR{h}", name=f"ktR{h}")
               for h in range(HPC)]
        vR = dram.tile([NKT, 128, 512], F32R, tag="vR", name="vR")
        xnR = [[dram.tile([128, 512], F32R, tag=f"xnR{h}_{p}", name=f"xnR{h}_{p}")
                for p in range(NQP)] for h in range(HPC)]
        recD = [[dram.tile([1, 512], F32, tag=f"recD{h}_{p}", name=f"recD{h}_{p}")
                 for p in range(NQP)] for h in range(HPC)]

        for _rep in range(REPS):
            QT = [big.tile([128, S], F32R, tag="qt", name="qt") for _ in range(HPC)]

            # ---------------- Phase A: projections + RoPE ----------------
            def rope_pair(tgt, cp):
                """tgt [128,1024] (chunk pair cp): tgt = tgt*cos + swap(tgt)*sin'."""
                sl = bass.ts(cp, 1024)
                tmp = work.tile([128, 1024], F32R, tag="ropetmp", bufs=2, name="ropetmp")
                nc.gpsimd.dma_start(tmp[0:64, :], tgt[64:128, :])
                nc.gpsimd.dma_start(tmp[64:128, :], tgt[0:64, :])
                nc.vector.tensor_mul(tmp, tmp, sin_sb[:, sl])
                nc.vector.tensor_mul(tgt, tgt, cos_sb[:, sl])
                nc.vector.tensor_add(tgt, tgt, tmp)

            first = True
            for which, xT_d, w_d, b_sb in (("q", xqT, wq, bq_sb), ("k", xkT, wk, bk_sb)):
                wt = []
                for cp in range(2):
                    xt = []
                    for d in range(ND):
                        if cp == 0:
                            t = wpool.tile([128, 512], PDT, tag="w", name="w")
                            nc.sync.dma_start(t, w_d[d * 128:(d + 1) * 128, :])
                            wt.append(t)
                        t = xpool.tile([128, 1024], PDT, tag="x", name="x")
                        nc.sync.dma_start(
                            t, xT_d[d * 128:(d + 1) * 128, cp * 1024:(cp + 1) * 1024])
                        xt.append(t)
                    if first:
                        nc.gpsimd.dma_start(cos_sb, cosT)
                        nc.gpsimd.dma_start(sin_sb, sinTs)
                        nc.gpsimd.dma_start(bvb_sb, bvb)
                        nc.gpsimd.dma_start(ones_sb, ones)
                        first = False
                    for h in range(HPC):
                        if which == "q":
                            tgt = QT[h][:, cp * 1024:(cp + 1) * 1024]
                        else:
                            tgt = work.tile([128, 1024], F32R, tag="kstage", bufs=4, name="kstage")
                        for half in range(2):
                            ps = psum.tile([128, 512], F32, tag="pa", bufs=PA_BUFS, name="pa")
                            for d in range(ND):
                                nc.tensor.matmul(
                                    ps,
                                    lhsT=wt[d][:, h * 128:(h + 1) * 128],
                                    rhs=xt[d][:, half * 512:(half + 1) * 512],
                                    start=(d == 0), stop=(d == ND - 1),
                                )
                            nc.scalar.activation(
                                tgt[:, half * 512:(half + 1) * 512], ps,
                                AF.Identity, bias=b_sb[:, h:h + 1])
                        rope_pair(tgt, cp)
                        if which == "k":
                            nc.gpsimd.dma_start(
                                ktR[h][:, cp * 1024:(cp + 1) * 1024], tgt)

            # V projection (natural [s, head*dk] layout)
            wvt = []
            for cp in range(2):
                xt = []
                for d in range(ND):
                    if cp == 0:
                        t = wpool.tile([128, 512], PDT, tag="w", name="w")
                        nc.sync.dma_start(t, wv[d * 128:(d + 1) * 128, :])
                        wvt.append(t)
                    t = xpool.tile([128, 1024], PDT, tag="x", name="x")
                    nc.sync.dma_start(
                        t, xvT[d * 128:(d + 1) * 128, cp * 1024:(cp + 1) * 1024])
                    xt.append(t)
                for sti in range(8):
                    st = cp * 8 + sti
                    ps = psum.tile([128, 512], F32, tag="pa", bufs=PA_BUFS, name="pa")
                    for d in range(ND):
                        nc.tensor.matmul(
                            ps,
                            lhsT=xt[d][:, sti * 128:(sti + 1) * 128],
                            rhs=wvt[d][:],
                            start=(d == 0), stop=(d == ND - 1),
                        )
                    vstage = work.tile([128, 512], F32R, tag="vstage", bufs=2, name="vstage")
                    nc.vector.tensor_add(vstage, ps, bvb_sb)
                    nc.sync.dma_start(vR[st, :, :], vstage)

            # ---------------- Phase B: attention per head ----------------
            for h in range(HPC):
                ktS = kv.tile([128, S], F32R, tag="ktS", name="ktS")
                for kh in range(2):
                    nc.sync.dma_start(
                        ktS[:, kh * 1024:(kh + 1) * 1024],
                        ktR[h][:, kh * 1024:(kh + 1) * 1024])
                vtS = kv.tile([128, S], F32R, tag="vtS", name="vtS")
                # gather head h columns from the st-major vR in quarter chunks
                for vq in range(4):
                    nc.sync.dma_start(
                        vtS[:, vq * 512:(vq + 1) * 512].rearrange(
                            "p (t c) -> p t c", t=4),
                        vR[vq * 4:(vq + 1) * 4, :, h * 128:(h + 1) * 128]
                        .rearrange("t p c -> p t c"))
                for p in range(NQP):
                    qsl = bass.ts(p, 512)
                    ps_x = psum.tile([128, 512], F32, tag="xun", bufs=XUN_BUFS, name="xun")
                    ps_d = psum.tile([1, 512], F32, tag="den", bufs=DEN_BUFS, name="den")
                    for kt in range(NKT):
                        ps_s = psum.tile([128, 512], F32, tag="sc", bufs=SC_BUFS, name="sc")
                        nc.tensor.matmul(
                            ps_s,
                            lhsT=ktS[:, kt * 128:(kt + 1) * 128],
                            rhs=QT[h][:, qsl],
                            start=True, stop=True,
                        )
                        ex = work.tile([128, 512], F32R, tag="expT", bufs=EXP_BUFS, name="expT")
                        nc.scalar.activation(ex, ps_s, AF.Exp, scale=SCALE)
                        nc.tensor.matmul(
                            ps_d,
                            lhsT=ones_sb[:],
                            rhs=ex[:],
                            start=(kt == 0), stop=(kt == NKT - 1),
                        )
                        nc.tensor.matmul(
                            ps_x,
                            lhsT=vtS[:, kt * 128:(kt + 1) * 128],
                            rhs=ex[:],
                            start=(kt == 0), stop=(kt == NKT - 1),
                        )
                    # softmax denominator: reciprocal + broadcast across partitions
                    denR = work.tile([1, 512], F32, tag="denR", bufs=1, name="denR")
                    nc.scalar.copy(denR, ps_d)
                    nc.vector.reciprocal(denR, denR)
                    nc.gpsimd.dma_start(recD[h][p][:], denR)
                    recB = work.tile([128, 512], F32, tag="recB", bufs=1, name="recB")
                    nc.gpsimd.dma_start(recB, _bcast_ap(recD[h][p][:], 128))
                    xn = work.tile([128, 512], F32R, tag="xn", bufs=2, name="xn")
                    nc.vector.tensor_mul(xn, ps_x, recB)
                    nc.sync.dma_start(xnR[h][p][:, :], xn)

            # ---------------- Phase C: output projection ----------------
            wot = []
            for h in range(HPC):
                for ncl in range(NNC):
                    t = wpool.tile([128, 512], F32R, tag="w", name="w")
                    nc.sync.dma_start(
                        t, wo[h * 128:(h + 1) * 128, ncl * 512:(ncl + 1) * 512])
                    wot.append(t)
            for qc in range(4):
                xnt = []
                for h in range(HPC):
                    t = xpool.tile([128, 512], F32R, tag="xn_in", bufs=XN_BUFS, name="xn_in")
                    nc.sync.dma_start(t, xnR[h][qc][:, :])
                    xnt.append(t)
                for qi in range(4):
                    qt = qc * 4 + qi
                    orow = [work.tile([128, 1024], F32, tag="kstage", bufs=4, name=f"orow{i}")
                            for i in range(2)]
                    for ncl in range(NNC):
                        _tg, _tb = ("pa", PA_BUFS) if ncl % 2 == 0 else ("sc", SC_BUFS)
                        ps = psum.tile([128, 512], F32, tag=_tg, bufs=_tb, name="pa")
                        for h in range(HPC):
                            nc.tensor.matmul(
                                ps,
                                lhsT=xnt[h][:, qi * 128:(qi + 1) * 128],
                                rhs=wot[h * NNC + ncl][:],
                                start=(h == 0), stop=(h == HPC - 1),
                            )
                        nc.vector.tensor_copy(
                            orow[ncl // 2][:, (ncl % 2) * 512:(ncl % 2 + 1) * 512], ps)
                    for half in range(2):
                        nc.sync.dma_start(
                            out[qt * 128:(qt + 1) * 128, half * 1024:(half + 1) * 1024],
                            orow[half])

    nc.finalize()
    return nc


def _rope_tables():
    inv_freq = (1.0 / (10000.0 ** (np.arange(0, DK, 2, dtype=np.float32) / DK))).astype(np.float32)
    pos = np.arange(S, dtype=np.float32)
    freqs = pos[:, None] * inv_freq[None, :]
    emb = np.concatenate([freqs, freqs], axis=-1)  # [S, DK]
    cos = np.cos(emb).astype(np.float32)
    sin = np.sin(emb).astype(np.float32)
    cosT = np.ascontiguousarray(cos.T)  # [DK, S]
    sinT = np.ascontiguousarray(sin.T)
    sinTs = sinT.copy()
    sinTs[0:64] *= -1.0
    return cosT, sinTs


def kernel(query, key, value, Wq, bq, Wk, bk, Wv, bv, Wo, bo):
    global _NC_CACHE, LAST_EXEC_NS
    query = np.asarray(query, dtype=np.float32)
    key = np.asarray(key, dtype=np.float32)
    value = np.asarray(value, dtype=np.float32)
    Wq = np.asarray(Wq, dtype=np.float32)
    Wk = np.asarray(Wk, dtype=np.float32)
    Wv = np.asarray(Wv, dtype=np.float32)
    Wo = np.asarray(Wo, dtype=np.float32)
    bq = np.asarray(bq, dtype=np.float32)
    bk = np.asarray(bk, dtype=np.float32)
    bv = np.asarray(bv, dtype=np.float32)
    bo = np.asarray(bo, dtype=np.float32)

    if _NC_CACHE is None:
        _NC_CACHE = build_nc()
    nc = _NC_CACHE

    cosT, sinTs = _rope_tables()
    WqT = np.ascontiguousarray(Wq.T)
    WkT = np.ascontiguousarray(Wk.T)
    WvT = np.ascontiguousarray(Wv.T)
    WoT = np.ascontiguousarray(Wo.T)
    pdt = ml_dtypes.bfloat16 if PROJ_BF16 else np.float32
    xT = {}
    for b in range(B):
        xT[("q", b)] = np.ascontiguousarray(query[b].T).astype(pdt)
        xT[("k", b)] = np.ascontiguousarray(key[b].T).astype(pdt)
        xT[("v", b)] = np.ascontiguousarray(value[b].T).astype(pdt)

    in_maps = []
    for c in range(8):
        b = c // 4
        g = c % 4
        cols = slice(g * 512, (g + 1) * 512)
        in_maps.append({
            "xqT": xT[("q", b)],
            "xkT": xT[("k", b)],
            "xvT": xT[("v", b)],
            "wq": np.ascontiguousarray(WqT[:, cols]).astype(pdt),
            "wk": np.ascontiguousarray(WkT[:, cols]).astype(pdt),
            "wv": np.ascontiguousarray(WvT[:, cols]).astype(pdt),
            "wo": np.ascontiguousarray(WoT[cols, :]),
            "bqT": np.ascontiguousarray(bq[cols].reshape(HPC, 128).T),
            "bkT": np.ascontiguousarray(bk[cols].reshape(HPC, 128).T),
            "bvb": np.ascontiguousarray(
                np.broadcast_to(bv[cols][None, :], (128, 512))),
            "cosT": cosT,
            "sinTs": sinTs,
            "ones": np.ones((128, 1), dtype=np.float32),
        })

    res = run_bass_kernel_spmd(nc, in_maps, core_ids=list(range(8)))
    LAST_EXEC_NS = res.exec_time_ns

    parts = [res.results[c]["out"] for c in range(8)]
    outp = np.empty((B, S, D), dtype=np.float32)
    for b in range(B):
        acc = parts[4 * b].astype(np.float32)
        for g in range(1, 4):
            acc = acc + parts[4 * b + g]
        outp[b] = acc + bo[None, :]
    return outp

